# revision 1
# baseline (speedup 1.0000x reference)
"""Trainium2 Bass kernel for nn_Encoder (R-GCN style message passing).

Math (faithful to the reference, including its s-major/f-major index mismatch):
    supports_ = concat_s(A[s] @ features)            # [N, S*F], cols k=s*F+f
    Vmat      = (W_comp @ W.transpose(1,0,2)).reshape(S*F, E)   # rows k=f*S+s
    out       = supports_ @ Vmat

Rewritten as one big contraction:
    Q_s[f, e]  = Vmat[s*F + f, e]        (contiguous 32-row block of Vmat)
    H_s        = features @ Q_s          # [N, E]  (tiny)
    out        = sum_s A[s] @ H_s
               = Hcat.T-contract over (s, m):  out.T = Hcat.T @ Acat
    where Acat[(s,m), n] = A[s, n, m]  (host-transposed shard)
          Hcat[(s,m), e] = H_s[m, e]

Sharding: node dim N split across 8 cores (1024 rows each). Each core
streams its 128 MiB A-shard through the PE as the moving operand with
H-chunks as 128x32 stationary weights, accumulating out.T [32, 1024] in
PSUM. Host does layout-only transforms (transpose/replicate/shard) and
the final gather+transpose; all arithmetic runs on device.
"""

import os
import numpy as np

import concourse.bass as bass
import concourse.mybir as mybir
from concourse import bacc, bass_utils
from concourse.tile import TileContext
from concourse.tile_rust import add_dep_helper

S, N, F, E = 4, 8192, 32, 32
P = 128
N_CORES = 8
NS = N // N_CORES          # 1024 node rows per core
KTOT = S * N               # 32768 contraction rows
NCHUNK = KTOT // P         # 256 K-chunks of 128
JPB = int(os.environ.get("KJPB", "4"))   # K-chunks per DMA block
NBLK = NCHUNK // JPB       # DMA blocks
MB = N // (P * JPB)        # DMA blocks per relation
MCH = N // P               # 64 m-chunks per relation

# Matmul dtype for the big streaming matmul ('f32' | 'f32r' | 'fp16' | 'bf16').
# fp16 halves the HBM traffic for A (the sole large operand) and runs the PE
# at full rate; measured scaled absmax error ~3e-4 vs the fp32 reference.
MAIN_DT = os.environ.get("KDT", "fp16")

_DT_MAP = {
    "f32": (mybir.dt.float32, np.float32),
    "f32r": (mybir.dt.float32r, np.float32),
    "fp16": (mybir.dt.float16, np.float16),
}


def _np_dt(key):
    if key == "bf16":
        import ml_dtypes
        return ml_dtypes.bfloat16
    return _DT_MAP[key][1]


def _build(dt_key):
    """Build + finalize the per-core Bass program (same program on all cores)."""
    if dt_key == "bf16":
        dt_main = mybir.dt.bfloat16
    else:
        dt_main = _DT_MAP[dt_key][0]
    f32 = mybir.dt.float32
    f32r = mybir.dt.float32r
    # H-phase dtype: match main dtype for 2-byte modes (hcat is quantized to
    # it anyway; enables FWL fast weight loads), f32r otherwise.
    dt_h = f32r
    defbufs = (6 if dt_key in ("fp16", "bf16") else 3) * 8 // JPB
    abufs = int(os.environ.get("KABUFS", str(defbufs)))

    nc = bacc.Bacc("TRN2")
    atc = nc.dram_tensor("atc", [KTOT, NS], dt_main, kind="ExternalInput")
    featT = nc.dram_tensor("featT", [F, N], dt_h, kind="ExternalInput")
    # per-relation expanded basis weights, all at base partition 0:
    # wmat[f, s*64 + b*32 + e] = W[b, (s*32+f)//4, e] replicated per Vmat row
    # wcs[f, s*2 + b] = W_comp[(s*32+f)%4, b]
    wmat = nc.dram_tensor("wmat", [F, S * 2 * E], f32, kind="ExternalInput")
    wcs = nc.dram_tensor("wcs", [F, S * 2], f32, kind="ExternalInput")
    outT = nc.dram_tensor("outT", [E, NS], f32, kind="ExternalOutput")

    # Contraction rows permuted so partition p's block data is one contiguous
    # run: row k = b*(P*JPB) + p*JPB + j  (16-32 KB per partition per DMA).
    atc_r = atc.rearrange("(b p j) n -> b p (j n)", p=P, j=JPB)

    with TileContext(nc) as tc:
        with (
            tc.tile_pool(name="consts", bufs=1) as consts,
            tc.tile_pool(name="hcatp", bufs=1) as hcatp,
            tc.tile_pool(name="abuf", bufs=abufs) as apool,
            tc.tile_pool(name="hps", bufs=4, space="PSUM") as hps,
            tc.tile_pool(name="ops", bufs=1, space="PSUM") as opsum,
            tc.tile_pool(name="osb", bufs=1) as osb,
        ):
            # A-block loads alternate between the two independent HWDGE rings
            # (SP/sync and ACT/scalar) to double descriptor-issue throughput.
            def a_dma(b, ab):
                eng = nc.sync if b % 2 == 0 else nc.scalar
                eng.dma_start(ab, atc_r[b])

            # ---- kick off the first A-block loads before anything else ----
            pre = {}
            for b in range(min(4, NBLK)):
                ab = apool.tile([P, JPB * NS], dt_main)
                a_dma(b, ab)
                pre[b] = ab

            # ---- constants ----
            ft = consts.tile([F, N], dt_h)
            nc.sync.dma_start(ft, featT[:, :])
            wm = consts.tile([F, S * 2 * E], f32)
            nc.sync.dma_start(wm, wmat[:, :])
            wc = consts.tile([F, S * 2], f32)
            nc.sync.dma_start(wc, wcs[:, :])

            # ---- Q_s [32, 32] per relation: Q_s = wc0*W0blk + wc1*W1blk
            tmp = consts.tile([F, E], f32)
            qs = []
            for s in range(S):
                q = consts.tile([F, E], f32, tag=f"q{s}")
                nc.vector.tensor_scalar_mul(
                    tmp, wm[:, s * 64 : s * 64 + E], wc[:, 2 * s : 2 * s + 1]
                )
                nc.vector.tensor_scalar_mul(
                    q, wm[:, s * 64 + E : (s + 1) * 64], wc[:, 2 * s + 1 : 2 * s + 2]
                )
                nc.vector.tensor_add(q, q, tmp)
                qr = consts.tile([F, E], dt_h, tag=f"qr{s}")
                nc.any.tensor_copy(qr, q)
                qs.append(qr)

            # ---- Hcat [128, NCHUNK*E]: chunk c (= s*MCH + mc) at cols c*E:(c+1)*E,
            #      Hcat_chunk[p, e] = sum_f featT[f, mc*P+p] * Q_s[f, e]
            hcat = hcatp.tile([P, NCHUNK * E], dt_main)

            def emit_h_block(bb, after=None):
                # all JPB chunks of block bb packed into one PSUM tile, one copy
                # block bb covers rows k = bb*(P*JPB) + p*JPB + j -> s = bb // MB,
                # m = (bb % MB)*P*JPB + p*JPB + j; ft is host-permuted to
                # [f, (g, j, p)] so the weight slice is contiguous.
                # `after` throttles scheduler run-ahead: without it the Tile
                # scheduler clusters all H matmuls, starving the A-block DMAs
                # of buffer slots mid-kernel.
                s, g = divmod(bb, MB)
                hp = hps.tile([P, JPB * E], f32)
                for j in range(JPB):
                    mm = nc.tensor.matmul(
                        hp[:, j * E : (j + 1) * E],
                        ft[:, (g * JPB + j) * P : (g * JPB + j + 1) * P],
                        qs[s],
                        start=True,
                        stop=True,
                    )
                    if after is not None:
                        add_dep_helper(
                            mm.ins, after.ins, sync=False,
                            reason="throttle H run-ahead",
                        )
                nc.any.tensor_copy(
                    hcat[:, bb * JPB * E : (bb + 1) * JPB * E], hp
                )

            # ---- main streaming matmul: out.T += Hcat_chunk.T @ A_block
            ps0 = opsum.tile([E, 512], f32)
            ps1 = opsum.tile([E, 512], f32)

            emit_h_block(0)
            mm_hist = []
            for b in range(NBLK):
                if b in pre:
                    ab = pre.pop(b)
                else:
                    ab = apool.tile([P, JPB * NS], dt_main)
                    a_dma(b, ab)
                if b + 1 < NBLK:
                    # anchor two blocks back: H(b+1) may overlap main(b-1) and
                    # main(b), so the H->hcat-copy->main-MM chain never sits on
                    # the PE critical path, while run-ahead stays bounded.
                    anchor = mm_hist[-2] if len(mm_hist) >= 2 else None
                    emit_h_block(b + 1, after=anchor)
                for j in range(JPB):
                    c = b * JPB + j
                    hc = hcat[:, c * E : (c + 1) * E]
                    first = c == 0
                    last = c == NCHUNK - 1
                    nc.tensor.matmul(
                        ps0, hc, ab[:, j * NS : j * NS + 512],
                        start=first, stop=last, skip_group_check=True,
                    )
                    mm = nc.tensor.matmul(
                        ps1, hc, ab[:, j * NS + 512 : (j + 1) * NS],
                        start=first, stop=last, skip_group_check=True,
                    )
                mm_hist.append(mm)

            # split output halves across engines + both HWDGE rings so the
            # ps0 half's copy+store overlaps the ps1 half's
            ot0 = osb.tile([E, 512], f32, tag="ot0")
            ot1 = osb.tile([E, 512], f32, tag="ot1")
            nc.scalar.copy(ot0, ps0)
            nc.vector.tensor_copy(ot1, ps1)
            nc.sync.dma_start(outT[:, 0:512], ot0)
            nc.scalar.dma_start(outT[:, 512:NS], ot1)

    nc.finalize()
    return nc


_built_cache = {}


def _get_nc(dt_key):
    if dt_key not in _built_cache:
        _built_cache[dt_key] = _build(dt_key)
    return _built_cache[dt_key]


def _shard_inputs(features, A, W, W_comp, dt_key):
    np_main = _np_dt(dt_key)
    features = np.asarray(features, dtype=np.float32)
    A = np.asarray(A, dtype=np.float32)
    W = np.asarray(W, dtype=np.float32)
    W_comp = np.asarray(W_comp, dtype=np.float32)

    # featT columns ordered (g, j, p) to match the permuted contraction rows
    featT = np.ascontiguousarray(
        features.reshape(MB, P, JPB, F).transpose(3, 0, 2, 1).reshape(F, N)
    ).astype(np.float32)
    wmat_full = np.concatenate(
        [np.repeat(W[0], S, axis=0), np.repeat(W[1], S, axis=0)], axis=1
    ).astype(np.float32)                                          # [128, 2E], row k
    wcs_full = np.stack(
        [np.tile(W_comp[:, 0], F), np.tile(W_comp[:, 1], F)], axis=1
    ).astype(np.float32)                                          # [128, 2]
    # regroup rows k = s*32+f into per-s column blocks at partitions f=0..31
    wmat = np.ascontiguousarray(
        wmat_full.reshape(S, F, 2 * E).transpose(1, 0, 2).reshape(F, S * 2 * E)
    )
    wcs = np.ascontiguousarray(
        wcs_full.reshape(S, F, 2).transpose(1, 0, 2).reshape(F, S * 2)
    )

    in_maps = []
    for c in range(N_CORES):
        a_sh = A[:, c * NS : (c + 1) * NS, :]                     # [S, NS, M]
        atc = np.ascontiguousarray(a_sh.transpose(0, 2, 1)).reshape(KTOT, NS)
        in_maps.append(
            {
                "atc": atc.astype(np_main),
                "featT": featT,
                "wmat": wmat,
                "wcs": wcs,
            }
        )
    return in_maps


def _run(features, A, W, W_comp, dt_key=None, trace=False):
    dt_key = dt_key or MAIN_DT
    nc = _get_nc(dt_key)
    in_maps = _shard_inputs(features, A, W, W_comp, dt_key)
    res = bass_utils.run_bass_kernel_spmd(
        nc, in_maps, core_ids=list(range(N_CORES)), trace=trace
    )
    out = np.concatenate(
        [res.results[c]["outT"].T for c in range(N_CORES)], axis=0
    ).astype(np.float32)
    return out, res


def kernel(features, A, W, W_comp):
    try:
        out, _ = _run(features, A, W, W_comp)
    except Exception:
        # Rare transient device-unrecoverable flakes: reset jax backends and
        # retry once with a freshly built program.
        import jax
        try:
            jax.clear_caches()
            jax.extend.backend.clear_backends()
        except Exception:
            pass
        _built_cache.clear()
        out, _ = _run(features, A, W, W_comp)
    return out

